# revision 22
# baseline (speedup 1.0000x reference)
"""Single-head attention layer on 8 TRN2 NeuronCores.

Data-parallel over batch: B=8, one batch element per core. Per core,
for x [S=2048, E=1024]:
    Q = x@Wq+bq; K = x@Wk+bk; V = x@Wv+bv        (KQ = VDIM = 128)
    O = softmax(Q K^T / sqrt(128)) V @ Wo + bo
All matmuls bf16 with f32 PSUM accumulation. Softmax skips the
max-subtraction (scores bounded for this input distribution).

v2 structure (vs the v1 baseline at ~151us profiled):
- PE warm-up matmuls at kernel start: HAM clock-gate releases (~3.4us of
  sustained normal MMs) before real work, so prep runs at 2.4 GHz, not
  1.2 (v1 was cold for its first 45us).
- Weights stream on sync/scalar/vector DMA queues in PARALLEL with x on
  gpsimd (v1 queued weights behind all 8.4MB of x: 12us tensor stall).
- Fused QKV: for each s-tile, transpose x chunks (PE), then use xT
  chunks as the stationary operand with [Wq|Wk|Wv] moving -> natural
  [s, 384] Q/K/V in one accumulation pass. V tiles feed the H matmul
  directly (v1 needed 16 xbar DMA-transposes); Q/K transpose back to
  [d, s] with 32 cheap PE transposes.
- Rowsum via 4 concurrent col-tiled MMs (tile_position=(0,32j), M=1)
  accumulating into partitions {0,32,64,96}: ~4x less PE time than
  v1's ones-stationary rowsum matmuls.
- exp on paired score tiles [128,1024] spanning 2 PSUM banks:
  (N+352)/1.2ns ACTIVATE overhead amortized (46us -> 37us scalar).
- Attention software pipeline: scores(g) | exp(g) | H(g-2)/rowsum(g-2),
  with chunk qq-1's output projection injected into chunk qq's stream
  and bias-adds alternating vector/gpsimd.
"""

import sys
from contextlib import ExitStack

for _p in ("/root/.axon_site", "/root/.axon_site/_ro/trn_rl_repo", "/opt/trn_rl_repo"):
    if _p not in sys.path:
        sys.path.append(_p)

import numpy as np

B, S, E = 8, 2048, 1024
KQ = 128
N_CORES = 8
S_TILES = S // 128          # 16
E_CHUNKS = E // 128         # 8
QC = 512                    # q columns per attention chunk
N_QCHUNKS = S // QC         # 4
N_GROUPS = S_TILES // 2     # 8 exp-groups (2 k-tiles each) per chunk
SCALE = float(1.0 / np.sqrt(KQ))


def build_nc():
    import concourse.bass as bass
    import concourse.tile as tile
    from concourse import bacc, mybir
    from concourse.masks import make_identity

    f32 = mybir.dt.float32
    bf16 = mybir.dt.bfloat16
    Exp = mybir.ActivationFunctionType.Exp

    nc = bacc.Bacc("TRN2", target_bir_lowering=False, debug=False,
                   num_devices=N_CORES)

    x_ext = nc.declare_dram_parameter("x", [S, E], f32, isOutput=False)
    wq_ext = nc.declare_dram_parameter("Wq", [E, KQ], f32, isOutput=False)
    bq_ext = nc.declare_dram_parameter("bq", [KQ], f32, isOutput=False)
    wk_ext = nc.declare_dram_parameter("Wk", [E, KQ], f32, isOutput=False)
    bk_ext = nc.declare_dram_parameter("bk", [KQ], f32, isOutput=False)
    wv_ext = nc.declare_dram_parameter("Wv", [E, KQ], f32, isOutput=False)
    bv_ext = nc.declare_dram_parameter("bv", [KQ], f32, isOutput=False)
    wo_ext = nc.declare_dram_parameter("Wo", [KQ, E], f32, isOutput=False)
    bo_ext = nc.declare_dram_parameter("bo", [E], f32, isOutput=False)
    out_ext = nc.declare_dram_parameter("out", [S, E], f32, isOutput=True)

    def bcast_ap(ap, parts, offset_elems, n):
        """Read AP replicating a DRAM row across `parts` partitions."""
        return bass.AP(
            tensor=ap.tensor,
            offset=ap.offset + offset_elems,
            ap=[[0, parts], [1, n]],
        )

    with tile.TileContext(nc) as tc, ExitStack() as ctx:
        singles = ctx.enter_context(tc.tile_pool(name="singles", bufs=1))
        xb_pool = ctx.enter_context(tc.tile_pool(name="xb", bufs=8))
        xt_pool = ctx.enter_context(tc.tile_pool(name="xt", bufs=4))
        p_pool = ctx.enter_context(tc.tile_pool(name="p", bufs=4))
        rs_pool = ctx.enter_context(tc.tile_pool(name="rs", bufs=2))
        o_pool = ctx.enter_context(tc.tile_pool(name="o", bufs=4))
        # PSUM: 8 banks of [128, 2KB]. ps_s 2x2 banks (scores pairs /
        # warmup / QKV prep), ps_h 2x1 (H accum / prep x-transposes ...
        # no: prep x-transposes live in ps_mm, qk-transposes in ps_r),
        # ps_r 1 (rowsum), ps_mm 1 (outproj / bcast / prep transposes).
        ps_s = ctx.enter_context(tc.tile_pool(name="ps_s", bufs=2, space="PSUM"))
        ps_h = ctx.enter_context(tc.tile_pool(name="ps_h", bufs=2, space="PSUM"))
        ps_r = ctx.enter_context(tc.tile_pool(name="ps_r", bufs=1, space="PSUM"))
        ps_mm = ctx.enter_context(tc.tile_pool(name="ps_mm", bufs=1, space="PSUM"))

        # ---- vector queue: constants needed by the tensor warm-up ----
        ones_big = singles.tile([128, 512], bf16)
        nc.vector.memset(ones_big[:], 1.0)
        ones_t = singles.tile([128, 1], bf16)
        nc.vector.memset(ones_t[:], 1.0)
        ones_row = singles.tile([1, 128], bf16)
        nc.vector.memset(ones_row[:], 1.0)

        # ---- weights first: sync+scalar queues get the HBM pipe ------
        # (cast-DMAs are gpsimd-only, so land f32 and cast via DVE;
        # separate tiles so each cast waits only its own DMA)
        wq_f32 = singles.tile([128, E_CHUNKS, 128], f32)
        nc.sync.dma_start(
            out=wq_f32[:],
            in_=wq_ext[:].rearrange("(p r) d -> p r d", r=8))
        wk_f32 = singles.tile([128, E_CHUNKS, 128], f32)
        nc.scalar.dma_start(
            out=wk_f32[:],
            in_=wk_ext[:].rearrange("(p r) d -> p r d", r=8))
        wv_f32 = singles.tile([128, E_CHUNKS, 128], f32)
        nc.sync.dma_start(
            out=wv_f32[:],
            in_=wv_ext[:].rearrange("(p r) d -> p r d", r=8))
        w_cat = singles.tile([128, E_CHUNKS, 384], bf16)   # [e%128, e//128, q|k|v]
        nc.vector.tensor_copy(w_cat[:, :, 0:128], wq_f32[:])
        nc.vector.tensor_copy(w_cat[:, :, 128:256], wk_f32[:])
        nc.vector.tensor_copy(w_cat[:, :, 256:384], wv_f32[:])

        b_cat_bc = singles.tile([128, 384], f32)
        nc.scalar.dma_start(out=b_cat_bc[:, 0:128], in_=bcast_ap(bq_ext[:], 128, 0, KQ))
        nc.scalar.dma_start(out=b_cat_bc[:, 128:256], in_=bcast_ap(bk_ext[:], 128, 0, KQ))
        nc.scalar.dma_start(out=b_cat_bc[:, 256:384], in_=bcast_ap(bv_ext[:], 128, 0, KQ))

        # ---- gpsimd queue: identity, then the x cast-DMA stream ------
        ident = singles.tile([128, 128], bf16)
        make_identity(nc, ident[:])

        # x DMAs are GATED behind the three weight DMAs (tiny gpsimd
        # reads of each w tile): HBM arbitration otherwise starves the
        # 2nd DMA of a queue once the x stream saturates the pipe
        # (measured: Wv 512KB took 20us to land beside x).
        w_gate = singles.tile([1, 4], f32)
        xb_tiles = []
        for g in range(8):
            xb = xb_pool.tile([128, 2, E], bf16, tag="xb", name=f"xb{g}")
            if g == 0:
                nc.gpsimd.tensor_copy(w_gate[:, 0:1], wq_f32[0:1, 0, 0:1])
                nc.gpsimd.tensor_copy(w_gate[:, 1:2], wk_f32[0:1, 0, 0:1])
                nc.gpsimd.tensor_copy(w_gate[:, 2:3], wv_f32[0:1, 0, 0:1])
            nc.gpsimd.dma_start(               # cast f32 -> bf16 in DMA
                out=xb[:],
                in_=x_ext[g * 256:(g + 1) * 256, :].rearrange(
                    "(c p) e -> p c e", p=128))
            xb_tiles.append(xb)

        # wo / bo ride gpsimd BEHIND the x stream (needed only ~mid-
        # attention; keeps them off the weight-critical window)
        wo_t = singles.tile([128, E], bf16)   # [v, e]
        nc.gpsimd.dma_start(out=wo_t[:], in_=wo_ext[:])
        bo_bc = singles.tile([128, E], f32)
        nc.gpsimd.dma_start(out=bo_bc[:], in_=bcast_ap(bo_ext[:], 128, 0, E))
        bo16_row = singles.tile([1, E], bf16)
        nc.gpsimd.dma_start(out=bo16_row[:], in_=bcast_ap(bo_ext[:], 1, 0, E))

        # ---- scalar: preload the exp ACT table off the critical path -
        exp_warm = singles.tile([128, 8], bf16)
        nc.scalar.activation(out=exp_warm[:], in_=ones_big[:, 0:8], func=Exp)

        # ---- tensor: HAM warm-up (normal MMs; transposes don't count) -
        for i in range(18):
            wu = ps_s.tile([128, 1024], f32, tag="s", name=f"wu{i}")
            nc.tensor.matmul(wu[:, 0:512], ones_big[:, 0:128], ones_big[:],
                             start=True, stop=True)

        # ---- prep pipeline: x-transpose -> fused QKV -> qk-transpose -
        qkv_sb = singles.tile([128, S_TILES, 384], bf16)   # [s%128, s//128, q|k|v]
        qT = singles.tile([128, S], bf16)
        kT = singles.tile([128, S], bf16)
        hT = singles.tile([128, S], bf16)

        def emit_x_transposes(t):
            xb = xb_tiles[t // 2]
            c = t % 2
            tp = ps_mm.tile([128, E_CHUNKS, 128], bf16, tag="mm", name=f"tp{t}")
            xbr = xb[:, c, :].rearrange("p (e r) -> p r e", r=8)
            for j in range(E_CHUNKS):
                nc.tensor.transpose(tp[:, j, :], xbr[:, j, :], ident[:])
            xt = xt_pool.tile([128, E_CHUNKS, 128], bf16, tag="xt", name=f"xt{t}")
            nc.vector.tensor_copy(xt[:], tp[:])
            return xt

        def emit_qkv(t, xt):
            qp = ps_s.tile([128, 1024], f32, tag="s", name=f"qkvps{t}")
            for j in range(E_CHUNKS):
                nc.tensor.matmul(qp[:, 0:384], xt[:, j, :], w_cat[:, j, :],
                                 start=(j == 0), stop=(j == E_CHUNKS - 1))
            nc.vector.tensor_add(qkv_sb[:, t, :], qp[:, 0:384], b_cat_bc[:])

        def emit_qk_transpose(t):
            tp = ps_r.tile([128, 2, 128], bf16, tag="r", name=f"qktp{t}")
            nc.tensor.transpose(tp[:, 0, :], qkv_sb[:, t, 0:128], ident[:])
            nc.tensor.transpose(tp[:, 1, :], qkv_sb[:, t, 128:256], ident[:])
            nc.vector.tensor_copy(qT[:, t * 128:(t + 1) * 128], tp[:, 0, :])
            nc.vector.tensor_copy(kT[:, t * 128:(t + 1) * 128], tp[:, 1, :])

        xts = {0: emit_x_transposes(0)}
        for t in range(S_TILES):
            if t + 1 < S_TILES:
                xts[t + 1] = emit_x_transposes(t + 1)
            emit_qkv(t, xts.pop(t))
            if t >= 1:
                emit_qk_transpose(t - 1)
        emit_qk_transpose(S_TILES - 1)

        def v_ap(t):
            return qkv_sb[:, t, 256:384]       # natural V tile [s_k, v]

        # ---- attention: 4 q-chunks, pipelined ------------------------
        def emit_outproj(qq, si):
            s0 = qq * QC + si * 128
            for half in range(2):
                o_ps = ps_mm.tile([128, 512], f32, tag="mm",
                                  name=f"ops{qq}_{si}_{half}")
                nc.tensor.matmul(o_ps[:, 0:512],
                                 hT[:, s0:s0 + 128],
                                 wo_t[:, half * 512:(half + 1) * 512],
                                 start=True, stop=True)
                o_sb = o_pool.tile([128, 512], f32, tag="o", name=f"osb{qq}_{si}_{half}")
                nc.vector.tensor_add(o_sb[:], o_ps[:, 0:512],
                                     bo_bc[:, half * 512:(half + 1) * 512])
                nc.sync.dma_start(
                    out=out_ext[s0:s0 + 128, half * 512:(half + 1) * 512],
                    in_=o_sb[:])

        def make_tail(qq, h_ps, r_ps):
            """Rowsum combine+broadcast (one K=4 matmul) + recip + hT
            normalize for chunk qq. Emitted inside the NEXT chunk's
            stream so the vector-chain wait hides under tensor work."""
            def tail():
                # partials live at partitions {0,32,64,96} of r_ps;
                # copy each to partition 0, free-dim side by side
                # single-partition [1,512] ops run one-lane (~670ns
                # each) - spread them across scalar/vector/gpsimd and
                # take the reciprocal AFTER broadcast (full-width 128-
                # lane op) so the vector FIFO stops serializing the
                # outproj stream behind this chain.
                CopyF = mybir.ActivationFunctionType.Copy
                r4 = rs_pool.tile([1, 4, 512], f32, tag="r4", name=f"r4_{qq}")
                for j in range(4):
                    if j % 2 == 0:
                        nc.scalar.activation(out=r4[:, j, :],
                                             in_=r_ps[32 * j:32 * j + 1, :],
                                             func=CopyF)
                    else:
                        nc.vector.tensor_copy(r4[:, j, :],
                                              r_ps[32 * j:32 * j + 1, :])
                r01 = rs_pool.tile([1, 512], f32, tag="r01")
                nc.gpsimd.tensor_add(r01[:], r4[:, 0, :], r4[:, 1, :])
                r23 = rs_pool.tile([1, 512], f32, tag="r23")
                nc.gpsimd.tensor_add(r23[:], r4[:, 2, :], r4[:, 3, :])
                rtot = rs_pool.tile([1, 512], bf16, tag="rtot")
                nc.gpsimd.tensor_add(rtot[:], r01[:], r23[:])
                rb_ps = ps_mm.tile([128, 512], f32, tag="mm", name=f"rb{qq}")
                nc.tensor.matmul(rb_ps[:], ones_row[:], rtot[:],
                                 start=True, stop=True)
                r_bc = rs_pool.tile([128, 512], f32, tag="r_bc")
                nc.vector.reciprocal_approx_fast(r_bc[:], rb_ps[:])
                for si in range(QC // 128):
                    sl = slice(si * 128, (si + 1) * 128)
                    nc.vector.tensor_mul(
                        hT[:, qq * QC + si * 128:qq * QC + (si + 1) * 128],
                        h_ps[:, sl], r_bc[:, sl])
            return tail

        pending_tail = None
        for qq in range(N_QCHUNKS):
            h_ps = ps_h.tile([128, QC], f32, tag="h", name=f"h{qq}")
            r_ps = ps_r.tile([128, QC], f32, tag="r", name=f"r{qq}")
            pgs = {}
            for g in range(N_GROUPS + 2):
                if g < N_GROUPS:
                    sg = ps_s.tile([128, 1024], f32, tag="s", name=f"sg{qq}_{g}")
                    for h2 in range(2):
                        t = 2 * g + h2
                        nc.tensor.matmul(sg[:, h2 * 512:(h2 + 1) * 512],
                                         kT[:, t * 128:(t + 1) * 128],
                                         qT[:, qq * QC:(qq + 1) * QC],
                                         start=True, stop=True)
                    pg = p_pool.tile([128, 1024], bf16, tag="p", name=f"p{qq}_{g}")
                    nc.scalar.activation(out=pg[:], in_=sg[:], func=Exp,
                                         scale=SCALE)
                    pgs[g] = pg
                if g == 1 and pending_tail is not None:
                    pending_tail()
                    pending_tail = None
                if g >= 2:
                    gg = g - 2
                    for h2 in range(2):
                        t = 2 * gg + h2
                        nc.tensor.matmul(h_ps[:],
                                         v_ap(t),
                                         pgs[gg][:, h2 * 512:(h2 + 1) * 512],
                                         start=(t == 0), stop=(t == S_TILES - 1))
                    if gg % 2 == 1:
                        # rowsum quad: 4 consecutive col-tiled MMs over
                        # tiles 2gg-2..2gg+1 run concurrently (4 XBUSes)
                        for dt in range(4):
                            t = 2 * gg - 2 + dt
                            nc.tensor.matmul(
                                r_ps[32 * dt:32 * dt + 1, :],
                                ones_t[:],
                                pgs[t // 2][:, (t % 2) * 512:(t % 2 + 1) * 512],
                                start=(gg == 1), stop=(gg == N_GROUPS - 1),
                                tile_position=(0, 32 * dt),
                                skip_group_check=True)
                        pgs.pop(gg - 1, None)
                        pgs.pop(gg, None)
                    if qq >= 1 and gg in (2, 4, 6, 7):
                        emit_outproj(qq - 1, {2: 0, 4: 1, 6: 2, 7: 3}[gg])
            pending_tail = make_tail(qq, h_ps, r_ps)

        # ---- flush: last chunk's tail + output projection ------------
        # bo folded in via a K=1 broadcast matmul so the PSUM->SBUF move
        # is a plain copy, alternating scalar/vector so both engines
        # drain in parallel; banks alternate ps_mm / ps_s.
        Copy = mybir.ActivationFunctionType.Copy
        pending_tail()
        for si in range(QC // 128):
            s0 = (N_QCHUNKS - 1) * QC + si * 128
            for half in range(2):
                use_s = (2 * si + half) % 2 == 1
                pool, tag, shape = ((ps_s, "s", [128, 1024]) if use_s
                                    else (ps_mm, "mm", [128, 512]))
                o_ps = pool.tile(shape, f32, tag=tag, name=f"fops{si}_{half}")
                nc.tensor.matmul(o_ps[:, 0:512], ones_row[:],
                                 bo16_row[:, half * 512:(half + 1) * 512],
                                 start=True, stop=False)
                nc.tensor.matmul(o_ps[:, 0:512],
                                 hT[:, s0:s0 + 128],
                                 wo_t[:, half * 512:(half + 1) * 512],
                                 start=False, stop=True)
                o_sb = o_pool.tile([128, 512], f32, tag="o", name=f"fosb{si}_{half}")
                if half == 0:
                    nc.scalar.activation(out=o_sb[:], in_=o_ps[:, 0:512],
                                         func=Copy)
                else:
                    nc.vector.tensor_copy(o_sb[:], o_ps[:, 0:512])
                nc.sync.dma_start(
                    out=out_ext[s0:s0 + 128, half * 512:(half + 1) * 512],
                    in_=o_sb[:])

    nc.compile()
    return nc


_NC = None


def kernel(**inputs):
    global _NC
    from concourse.bass_utils import run_bass_kernel_spmd

    if _NC is None:
        _NC = build_nc()

    x = np.asarray(inputs["embedding_matrix"], dtype=np.float32)
    shared = {k: np.ascontiguousarray(np.asarray(inputs[k], dtype=np.float32))
              for k in ("Wq", "bq", "Wk", "bk", "Wv", "bv", "Wo", "bo")}
    in_maps = [dict(shared, x=np.ascontiguousarray(x[c])) for c in range(N_CORES)]

    res = run_bass_kernel_spmd(_NC, in_maps, core_ids=list(range(N_CORES)))
    out = np.stack([res.results[c]["out"] for c in range(N_CORES)], axis=0)
    return out.astype(np.float32)


# revision 24
# speedup vs baseline: 1.1834x; 1.1834x over previous
"""Single-head attention layer on 8 TRN2 NeuronCores.

Data-parallel over batch: B=8, one batch element per core. Per core,
for x [S=2048, E=1024]:
    Q = x@Wq+bq; K = x@Wk+bk; V = x@Wv+bv        (KQ = VDIM = 128)
    O = softmax(Q K^T / sqrt(128)) V @ Wo + bo
All matmuls bf16 with f32 PSUM accumulation. Softmax skips the
max-subtraction (scores bounded for this input distribution).

v2 structure (vs the v1 baseline at ~151us profiled):
- PE warm-up matmuls at kernel start: HAM clock-gate releases (~3.4us of
  sustained normal MMs) before real work, so prep runs at 2.4 GHz, not
  1.2 (v1 was cold for its first 45us).
- Weights stream on sync/scalar/vector DMA queues in PARALLEL with x on
  gpsimd (v1 queued weights behind all 8.4MB of x: 12us tensor stall).
- Fused QKV: for each s-tile, transpose x chunks (PE), then use xT
  chunks as the stationary operand with [Wq|Wk|Wv] moving -> natural
  [s, 384] Q/K/V in one accumulation pass. V tiles feed the H matmul
  directly (v1 needed 16 xbar DMA-transposes); Q/K transpose back to
  [d, s] with 32 cheap PE transposes.
- Rowsum via 4 concurrent col-tiled MMs (tile_position=(0,32j), M=1)
  accumulating into partitions {0,32,64,96}: ~4x less PE time than
  v1's ones-stationary rowsum matmuls.
- exp on paired score tiles [128,1024] spanning 2 PSUM banks:
  (N+352)/1.2ns ACTIVATE overhead amortized (46us -> 37us scalar).
- Attention software pipeline: scores(g) | exp(g) | H(g-2)/rowsum(g-2),
  with chunk qq-1's output projection injected into chunk qq's stream
  and bias-adds alternating vector/gpsimd.
"""

import sys
from contextlib import ExitStack

for _p in ("/root/.axon_site", "/root/.axon_site/_ro/trn_rl_repo", "/opt/trn_rl_repo"):
    if _p not in sys.path:
        sys.path.append(_p)

import numpy as np

B, S, E = 8, 2048, 1024
KQ = 128
N_CORES = 8
S_TILES = S // 128          # 16
E_CHUNKS = E // 128         # 8
QC = 512                    # q columns per attention chunk
N_QCHUNKS = S // QC         # 4
N_GROUPS = S_TILES // 2     # 8 exp-groups (2 k-tiles each) per chunk
SCALE = float(1.0 / np.sqrt(KQ))


def build_nc():
    import concourse.bass as bass
    import concourse.tile as tile
    from concourse import bacc, mybir
    from concourse.masks import make_identity

    f32 = mybir.dt.float32
    bf16 = mybir.dt.bfloat16
    Exp = mybir.ActivationFunctionType.Exp

    nc = bacc.Bacc("TRN2", target_bir_lowering=False, debug=False,
                   num_devices=N_CORES)

    x_ext = nc.declare_dram_parameter("x", [S, E], f32, isOutput=False)
    wq_ext = nc.declare_dram_parameter("Wq", [E, KQ], f32, isOutput=False)
    bq_ext = nc.declare_dram_parameter("bq", [KQ], f32, isOutput=False)
    wk_ext = nc.declare_dram_parameter("Wk", [E, KQ], f32, isOutput=False)
    bk_ext = nc.declare_dram_parameter("bk", [KQ], f32, isOutput=False)
    wv_ext = nc.declare_dram_parameter("Wv", [E, KQ], f32, isOutput=False)
    bv_ext = nc.declare_dram_parameter("bv", [KQ], f32, isOutput=False)
    wo_ext = nc.declare_dram_parameter("Wo", [KQ, E], f32, isOutput=False)
    bo_ext = nc.declare_dram_parameter("bo", [E], f32, isOutput=False)
    out_ext = nc.declare_dram_parameter("out", [S, E], f32, isOutput=True)

    def bcast_ap(ap, parts, offset_elems, n):
        """Read AP replicating a DRAM row across `parts` partitions."""
        return bass.AP(
            tensor=ap.tensor,
            offset=ap.offset + offset_elems,
            ap=[[0, parts], [1, n]],
        )

    with tile.TileContext(nc) as tc, ExitStack() as ctx:
        singles = ctx.enter_context(tc.tile_pool(name="singles", bufs=1))
        xb_pool = ctx.enter_context(tc.tile_pool(name="xb", bufs=8))
        xt_pool = ctx.enter_context(tc.tile_pool(name="xt", bufs=4))
        p_pool = ctx.enter_context(tc.tile_pool(name="p", bufs=4))
        rs_pool = ctx.enter_context(tc.tile_pool(name="rs", bufs=2))
        o_pool = ctx.enter_context(tc.tile_pool(name="o", bufs=4))
        # PSUM: 8 banks of [128, 2KB]. ps_s 2x2 banks (scores pairs /
        # warmup / QKV prep), ps_h 2x1 (H accum / prep x-transposes ...
        # no: prep x-transposes live in ps_mm, qk-transposes in ps_r),
        # ps_r 1 (rowsum), ps_mm 1 (outproj / bcast / prep transposes).
        ps_s = ctx.enter_context(tc.tile_pool(name="ps_s", bufs=2, space="PSUM"))
        ps_h = ctx.enter_context(tc.tile_pool(name="ps_h", bufs=2, space="PSUM"))
        ps_r = ctx.enter_context(tc.tile_pool(name="ps_r", bufs=1, space="PSUM"))
        ps_mm = ctx.enter_context(tc.tile_pool(name="ps_mm", bufs=1, space="PSUM"))

        # ---- vector queue: constants needed by the tensor warm-up ----
        ones_big = singles.tile([128, 512], bf16)
        nc.vector.memset(ones_big[:], 1.0)
        ones_t = singles.tile([128, 1], bf16)
        nc.vector.memset(ones_t[:], 1.0)
        ones_row = singles.tile([1, 128], bf16)
        nc.vector.memset(ones_row[:], 1.0)

        # ---- weights first: sync+scalar queues get the HBM pipe ------
        # (cast-DMAs are gpsimd-only, so land f32 and cast via DVE;
        # separate tiles so each cast waits only its own DMA)
        wq_f32 = singles.tile([128, E_CHUNKS, 128], f32)
        nc.sync.dma_start(
            out=wq_f32[:],
            in_=wq_ext[:].rearrange("(p r) d -> p r d", r=8))
        wk_f32 = singles.tile([128, E_CHUNKS, 128], f32)
        nc.scalar.dma_start(
            out=wk_f32[:],
            in_=wk_ext[:].rearrange("(p r) d -> p r d", r=8))
        wv_f32 = singles.tile([128, E_CHUNKS, 128], f32)
        nc.sync.dma_start(
            out=wv_f32[:],
            in_=wv_ext[:].rearrange("(p r) d -> p r d", r=8))
        w_cat = singles.tile([128, E_CHUNKS, 384], bf16)   # [e%128, e//128, q|k|v]
        nc.vector.tensor_copy(w_cat[:, :, 0:128], wq_f32[:])
        nc.vector.tensor_copy(w_cat[:, :, 128:256], wk_f32[:])
        nc.vector.tensor_copy(w_cat[:, :, 256:384], wv_f32[:])

        b_cat_bc = singles.tile([128, 384], f32)
        nc.scalar.dma_start(out=b_cat_bc[:, 0:128], in_=bcast_ap(bq_ext[:], 128, 0, KQ))
        nc.scalar.dma_start(out=b_cat_bc[:, 128:256], in_=bcast_ap(bk_ext[:], 128, 0, KQ))
        nc.scalar.dma_start(out=b_cat_bc[:, 256:384], in_=bcast_ap(bv_ext[:], 128, 0, KQ))

        # ---- gpsimd queue: identity, then the x cast-DMA stream ------
        ident = singles.tile([128, 128], bf16)
        make_identity(nc, ident[:])

        # x DMAs are GATED behind the three weight DMAs (tiny gpsimd
        # reads of each w tile): HBM arbitration otherwise starves the
        # 2nd DMA of a queue once the x stream saturates the pipe
        # (measured: Wv 512KB took 20us to land beside x).
        w_gate = singles.tile([1, 4], f32)
        xb_tiles = []
        for g in range(8):
            xb = xb_pool.tile([128, 2, E], bf16, tag="xb", name=f"xb{g}")
            if g == 0:
                nc.gpsimd.tensor_copy(w_gate[:, 0:1], wq_f32[0:1, 0, 0:1])
                nc.gpsimd.tensor_copy(w_gate[:, 1:2], wk_f32[0:1, 0, 0:1])
                nc.gpsimd.tensor_copy(w_gate[:, 2:3], wv_f32[0:1, 0, 0:1])
            nc.gpsimd.dma_start(               # cast f32 -> bf16 in DMA
                out=xb[:],
                in_=x_ext[g * 256:(g + 1) * 256, :].rearrange(
                    "(c p) e -> p c e", p=128))
            xb_tiles.append(xb)

        # wo / bo ride gpsimd BEHIND the x stream (needed only ~mid-
        # attention; keeps them off the weight-critical window)
        wo_t = singles.tile([128, E], bf16)   # [v, e]
        nc.gpsimd.dma_start(out=wo_t[:], in_=wo_ext[:])
        bo_bc = singles.tile([128, E], f32)
        nc.gpsimd.dma_start(out=bo_bc[:], in_=bcast_ap(bo_ext[:], 128, 0, E))
        bo16_row = singles.tile([1, E], bf16)
        nc.gpsimd.dma_start(out=bo16_row[:], in_=bcast_ap(bo_ext[:], 1, 0, E))

        # ---- scalar: preload the exp ACT table off the critical path -
        exp_warm = singles.tile([128, 8], bf16)
        nc.scalar.activation(out=exp_warm[:], in_=ones_big[:, 0:8], func=Exp)

        # ---- tensor: HAM warm-up (normal MMs; transposes don't count) -
        for i in range(26):
            wu = ps_s.tile([128, 1024], f32, tag="s", name=f"wu{i}")
            nc.tensor.matmul(wu[:, 0:512], ones_big[:, 0:128], ones_big[:],
                             start=True, stop=True)

        # ---- prep pipeline: x-transpose -> fused QKV -> qk-transpose -
        qkv_sb = singles.tile([128, S_TILES, 384], bf16)   # [s%128, s//128, q|k|v]
        qT = singles.tile([128, S], bf16)
        kT = singles.tile([128, S], bf16)
        hT = singles.tile([128, S], bf16)

        def emit_x_transposes(t):
            xb = xb_tiles[t // 2]
            c = t % 2
            tp = ps_mm.tile([128, E_CHUNKS, 128], bf16, tag="mm", name=f"tp{t}")
            xbr = xb[:, c, :].rearrange("p (e r) -> p r e", r=8)
            for j in range(E_CHUNKS):
                nc.tensor.transpose(tp[:, j, :], xbr[:, j, :], ident[:])
            xt = xt_pool.tile([128, E_CHUNKS, 128], bf16, tag="xt", name=f"xt{t}")
            nc.vector.tensor_copy(xt[:], tp[:])
            return xt

        def emit_qkv(t, xt):
            qp = ps_s.tile([128, 1024], f32, tag="s", name=f"qkvps{t}")
            for j in range(E_CHUNKS):
                nc.tensor.matmul(qp[:, 0:384], xt[:, j, :], w_cat[:, j, :],
                                 start=(j == 0), stop=(j == E_CHUNKS - 1))
            nc.vector.tensor_add(qkv_sb[:, t, :], qp[:, 0:384], b_cat_bc[:])

        def emit_qk_transpose(t):
            tp = ps_r.tile([128, 2, 128], bf16, tag="r", name=f"qktp{t}")
            nc.tensor.transpose(tp[:, 0, :], qkv_sb[:, t, 0:128], ident[:])
            nc.tensor.transpose(tp[:, 1, :], qkv_sb[:, t, 128:256], ident[:])
            nc.vector.tensor_copy(qT[:, t * 128:(t + 1) * 128], tp[:, 0, :])
            nc.vector.tensor_copy(kT[:, t * 128:(t + 1) * 128], tp[:, 1, :])

        xts = {0: emit_x_transposes(0)}
        for t in range(S_TILES):
            if t + 1 < S_TILES:
                xts[t + 1] = emit_x_transposes(t + 1)
            emit_qkv(t, xts.pop(t))
            if t >= 1:
                emit_qk_transpose(t - 1)
        emit_qk_transpose(S_TILES - 1)

        def v_ap(t):
            return qkv_sb[:, t, 256:384]       # natural V tile [s_k, v]

        # ---- attention: 4 q-chunks, pipelined ------------------------
        def emit_outproj(qq, si):
            s0 = qq * QC + si * 128
            for half in range(2):
                o_ps = ps_mm.tile([128, 512], f32, tag="mm",
                                  name=f"ops{qq}_{si}_{half}")
                nc.tensor.matmul(o_ps[:, 0:512],
                                 hT[:, s0:s0 + 128],
                                 wo_t[:, half * 512:(half + 1) * 512],
                                 start=True, stop=True)
                o_sb = o_pool.tile([128, 512], f32, tag="o", name=f"osb{qq}_{si}_{half}")
                nc.vector.tensor_add(o_sb[:], o_ps[:, 0:512],
                                     bo_bc[:, half * 512:(half + 1) * 512])
                nc.sync.dma_start(
                    out=out_ext[s0:s0 + 128, half * 512:(half + 1) * 512],
                    in_=o_sb[:])

        def make_tail(qq, h_ps, r_ps):
            """Rowsum combine+broadcast (one K=4 matmul) + recip + hT
            normalize for chunk qq. Emitted inside the NEXT chunk's
            stream so the vector-chain wait hides under tensor work."""
            def tail():
                # partials live at partitions {0,32,64,96} of r_ps;
                # copy each to partition 0, free-dim side by side
                r4 = rs_pool.tile([1, 4, 512], f32, tag="r4", name=f"r4_{qq}")
                for j in range(4):
                    nc.vector.tensor_copy(r4[:, j, :],
                                          r_ps[32 * j:32 * j + 1, :])
                r01 = rs_pool.tile([1, 512], f32, tag="r01")
                nc.vector.tensor_add(r01[:], r4[:, 0, :], r4[:, 1, :])
                r23 = rs_pool.tile([1, 512], f32, tag="r23")
                nc.vector.tensor_add(r23[:], r4[:, 2, :], r4[:, 3, :])
                rtot = rs_pool.tile([1, 512], f32, tag="rtot")
                nc.vector.tensor_add(rtot[:], r01[:], r23[:])
                rinv = rs_pool.tile([1, 512], f32, tag="rinv")
                nc.vector.reciprocal_approx_fast(rinv[:], rtot[:])
                r16 = rs_pool.tile([1, 512], bf16, tag="r16")
                nc.vector.tensor_copy(r16[:], rinv[:])
                rb_ps = ps_mm.tile([128, 512], f32, tag="mm", name=f"rb{qq}")
                nc.tensor.matmul(rb_ps[:], ones_row[:], r16[:],
                                 start=True, stop=True)
                r_bc = rs_pool.tile([128, 512], f32, tag="r_bc")
                nc.vector.tensor_copy(r_bc[:], rb_ps[:])
                for si in range(QC // 128):
                    sl = slice(si * 128, (si + 1) * 128)
                    nc.vector.tensor_mul(
                        hT[:, qq * QC + si * 128:qq * QC + (si + 1) * 128],
                        h_ps[:, sl], r_bc[:, sl])
            return tail

        pending_tail = None
        for qq in range(N_QCHUNKS):
            h_ps = ps_h.tile([128, QC], f32, tag="h", name=f"h{qq}")
            r_ps = ps_r.tile([128, QC], f32, tag="r", name=f"r{qq}")
            pgs = {}
            for g in range(N_GROUPS + 2):
                if g < N_GROUPS:
                    sg = ps_s.tile([128, 1024], f32, tag="s", name=f"sg{qq}_{g}")
                    for h2 in range(2):
                        t = 2 * g + h2
                        nc.tensor.matmul(sg[:, h2 * 512:(h2 + 1) * 512],
                                         kT[:, t * 128:(t + 1) * 128],
                                         qT[:, qq * QC:(qq + 1) * QC],
                                         start=True, stop=True)
                    pg = p_pool.tile([128, 1024], bf16, tag="p", name=f"p{qq}_{g}")
                    nc.scalar.activation(out=pg[:], in_=sg[:], func=Exp,
                                         scale=SCALE)
                    pgs[g] = pg
                if g == 1 and pending_tail is not None:
                    pending_tail()
                    pending_tail = None
                if g >= 2:
                    gg = g - 2
                    for h2 in range(2):
                        t = 2 * gg + h2
                        nc.tensor.matmul(h_ps[:],
                                         v_ap(t),
                                         pgs[gg][:, h2 * 512:(h2 + 1) * 512],
                                         start=(t == 0), stop=(t == S_TILES - 1))
                    if gg % 2 == 1:
                        # rowsum quad: 4 consecutive col-tiled MMs over
                        # tiles 2gg-2..2gg+1 run concurrently (4 XBUSes)
                        for dt in range(4):
                            t = 2 * gg - 2 + dt
                            nc.tensor.matmul(
                                r_ps[32 * dt:32 * dt + 1, :],
                                ones_t[:],
                                pgs[t // 2][:, (t % 2) * 512:(t % 2 + 1) * 512],
                                start=(gg == 1), stop=(gg == N_GROUPS - 1),
                                tile_position=(0, 32 * dt),
                                skip_group_check=True)
                        pgs.pop(gg - 1, None)
                        pgs.pop(gg, None)
                    if qq >= 1 and gg in (2, 4, 6, 7):
                        emit_outproj(qq - 1, {2: 0, 4: 1, 6: 2, 7: 3}[gg])
            pending_tail = make_tail(qq, h_ps, r_ps)

        # ---- flush: last chunk's tail + output projection ------------
        # bo folded in via a K=1 broadcast matmul so the PSUM->SBUF move
        # is a plain copy, alternating scalar/vector so both engines
        # drain in parallel; banks alternate ps_mm / ps_s.
        Copy = mybir.ActivationFunctionType.Copy
        pending_tail()
        for si in range(QC // 128):
            s0 = (N_QCHUNKS - 1) * QC + si * 128
            for half in range(2):
                use_s = (2 * si + half) % 2 == 1
                pool, tag, shape = ((ps_s, "s", [128, 1024]) if use_s
                                    else (ps_mm, "mm", [128, 512]))
                o_ps = pool.tile(shape, f32, tag=tag, name=f"fops{si}_{half}")
                nc.tensor.matmul(o_ps[:, 0:512], ones_row[:],
                                 bo16_row[:, half * 512:(half + 1) * 512],
                                 start=True, stop=False)
                nc.tensor.matmul(o_ps[:, 0:512],
                                 hT[:, s0:s0 + 128],
                                 wo_t[:, half * 512:(half + 1) * 512],
                                 start=False, stop=True)
                o_sb = o_pool.tile([128, 512], f32, tag="o", name=f"fosb{si}_{half}")
                if half == 0:
                    nc.scalar.activation(out=o_sb[:], in_=o_ps[:, 0:512],
                                         func=Copy)
                else:
                    nc.vector.tensor_copy(o_sb[:], o_ps[:, 0:512])
                nc.sync.dma_start(
                    out=out_ext[s0:s0 + 128, half * 512:(half + 1) * 512],
                    in_=o_sb[:])

    nc.compile()
    return nc


_NC = None


def kernel(**inputs):
    global _NC
    from concourse.bass_utils import run_bass_kernel_spmd

    if _NC is None:
        _NC = build_nc()

    x = np.asarray(inputs["embedding_matrix"], dtype=np.float32)
    shared = {k: np.ascontiguousarray(np.asarray(inputs[k], dtype=np.float32))
              for k in ("Wq", "bq", "Wk", "bk", "Wv", "bv", "Wo", "bo")}
    in_maps = [dict(shared, x=np.ascontiguousarray(x[c])) for c in range(N_CORES)]

    res = run_bass_kernel_spmd(_NC, in_maps, core_ids=list(range(N_CORES)))
    out = np.stack([res.results[c]["out"] for c in range(N_CORES)], axis=0)
    return out.astype(np.float32)
